# revision 4
# baseline (speedup 1.0000x reference)
"""Trainium2 Bass kernel for nn_LocalDecoder (ConvONet LocalDecoder: trilinear
grid sample + 5-block ResNet MLP decoder).

Strategy (8 NeuronCores):
  - Data-parallel over points: cores 0-3 take batch 0, cores 4-7 take batch 1,
    16384 points per core.
  - The feature grid is repacked on the host into an 8-shift 2x2x2-block table
    [8*32^3, 8*128] fp16: row (s, bz, by, bx) holds the 2x2x2 voxel block at
    alignment-shift s = (sz, sy, sx).  Every query point's 8 trilinear corners
    are then exactly ONE 2KB row -> one indirect-DMA descriptor per point.
  - Device computes voxel indices + trilinear weights on VectorE, gathers
    point-blocks via gpsimd indirect DMA (128 points/call), interpolates with
    fused scalar_tensor_tensor ops, transposes [pts,ch]->[ch,pts] on TensorE,
    and runs the MLP in fp16 with the residual stream resident in PSUM
    (fc_c / b1 matmuls accumulate in place; biases folded into ACT relu views).
"""

import numpy as np

import concourse.bass as bass
import concourse.bacc as bacc
import concourse.mybir as mybir
import concourse.tile as tile
from concourse.bass_utils import run_bass_kernel_spmd
from concourse.masks import make_identity

# ---- problem constants (hardcoded per contract) ----
B, N, R = 2, 65536, 64
C = 128            # grid feature channels
H = 256            # MLP hidden
NB = 5             # resnet blocks
PADDING = 0.1

NCORES = 8
CPB = NCORES // B          # cores per batch = 4
NPTS = N // CPB            # points per core = 16384
P = 128                    # partitions
T = NPTS // P              # 128 point-tiles of 128 per core
TPC = 4                    # tiles per chunk (chunk = 512 points)
NCH = T // TPC             # 32 chunks
NF = TPC * P               # chunk free dim = 512
VB = 32 * 32 * 32          # blocks per shift copy
V8 = 8 * VB                # table rows
ROW = 8 * C                # fp16 elems per table row (2KB)

SCALE = float(np.float32(63.0) / np.float32(1.0 + PADDING + 1e-3))
OFF = 31.5

F16 = mybir.dt.float16
F32 = mybir.dt.float32
I32 = mybir.dt.int32
ALU = mybir.AluOpType
AF = mybir.ActivationFunctionType

_CACHE = {}


def _build_nc():
    nc = bacc.Bacc("TRN2", target_bir_lowering=False, debug=False)

    table = nc.dram_tensor("table", [V8, ROW], F16, kind="ExternalInput")
    pts = nc.dram_tensor("pts", [NPTS, 3], F32, kind="ExternalInput")
    ptpad = nc.dram_tensor("ptpad", [4, NPTS], F16, kind="ExternalInput")
    fcp = nc.dram_tensor("fcp", [4, H], F16, kind="ExternalInput")
    wc = nc.dram_tensor("wc", [NB, C, H], F16, kind="ExternalInput")
    b0w = nc.dram_tensor("b0w", [NB, 2, P, H], F16, kind="ExternalInput")
    b1w = nc.dram_tensor("b1w", [NB, 2, P, H], F16, kind="ExternalInput")
    oww = nc.dram_tensor("oww", [P, 2], F16, kind="ExternalInput")
    rb = nc.dram_tensor("rb", [P, 12], F32, kind="ExternalInput")
    b0b = nc.dram_tensor("b0b", [P, 10], F32, kind="ExternalInput")
    outb = nc.dram_tensor("outb", [1, 1], F32, kind="ExternalInput")
    out_dev = nc.dram_tensor("out_dev", [1, NPTS], F32, kind="ExternalOutput")

    with tile.TileContext(nc) as tc:
        with (
            tc.tile_pool(name="const", bufs=1) as kpool,
            tc.tile_pool(name="gather", bufs=4) as gpool,
            tc.tile_pool(name="feat", bufs=3) as fpool,
            tc.tile_pool(name="cs", bufs=2) as cpool,
            tc.tile_pool(name="act", bufs=3) as spool,
            tc.tile_pool(name="net_ps", bufs=4, space="PSUM") as npool,
            tc.tile_pool(name="h_ps", bufs=2, space="PSUM") as hpool,
            tc.tile_pool(name="tr_ps", bufs=1, space="PSUM") as trpool,
            tc.tile_pool(name="o_ps", bufs=1, space="PSUM") as opool,
        ):
            # ---------- load constants ----------
            fcp_sb = kpool.tile([4, H], F16, tag="fcp")
            nc.sync.dma_start(fcp_sb[:], fcp[:])
            wc_sb = []
            b0_sb = []
            b1_sb = []
            for i in range(NB):
                w = kpool.tile([P, H], F16, tag=f"wc{i}")
                nc.sync.dma_start(w[:], wc[i, :, :])
                wc_sb.append(w)
                r0 = []
                r1 = []
                for kk in range(2):
                    a = kpool.tile([P, H], F16, tag=f"b0_{i}_{kk}")
                    nc.sync.dma_start(a[:], b0w[i, kk, :, :])
                    r0.append(a)
                    b = kpool.tile([P, H], F16, tag=f"b1_{i}_{kk}")
                    nc.sync.dma_start(b[:], b1w[i, kk, :, :])
                    r1.append(b)
                b0_sb.append(r0)
                b1_sb.append(r1)
            ow_sb = kpool.tile([P, 2], F16, tag="oww")
            nc.sync.dma_start(ow_sb[:], oww[:])
            rb_sb = kpool.tile([P, 12], F32, tag="rb")
            nc.sync.dma_start(rb_sb[:], rb[:])
            b0b_sb = kpool.tile([P, 10], F32, tag="b0b")
            nc.sync.dma_start(b0b_sb[:], b0b[:])
            outb_sb = kpool.tile([1, 1], F32, tag="outb")
            nc.sync.dma_start(outb_sb[:], outb[:])
            ptp_sb = kpool.tile([4, NPTS], F16, tag="ptp")
            nc.sync.dma_start(ptp_sb[:], ptpad[:])

            ident = kpool.tile([P, P], F16, tag="ident")
            make_identity(nc, ident[:])

            out_acc = kpool.tile([1, NPTS], F32, tag="oacc")

            # ---------- index / weight precompute (VectorE) ----------
            # p_slab[p, t*3+c] = pts[p*T + t, c]
            p_slab = kpool.tile([P, T * 3], F32, tag="pslab")
            nc.sync.dma_start(
                p_slab[:], pts[:].rearrange("(p t) c -> p (t c)", p=P)
            )
            ix = kpool.tile([P, T * 3], F32, tag="ix")
            nc.vector.tensor_scalar(ix[:], p_slab[:], SCALE, OFF, op0=ALU.mult, op1=ALU.add)
            nc.vector.tensor_scalar_max(ix[:], ix[:], 0.0)
            nc.vector.tensor_scalar_min(ix[:], ix[:], 63.0)
            x0i = kpool.tile([P, T * 3], I32, tag="x0i")
            nc.vector.tensor_copy(x0i[:], ix[:])
            x0f = kpool.tile([P, T * 3], F32, tag="x0f")
            nc.vector.tensor_copy(x0f[:], x0i[:])
            fixm = kpool.tile([P, T * 3], F32, tag="fixm")
            nc.vector.tensor_tensor(out=fixm[:], in0=x0f[:], in1=ix[:], op=ALU.is_gt)
            nc.vector.tensor_tensor(out=x0f[:], in0=x0f[:], in1=fixm[:], op=ALU.subtract)
            nc.vector.tensor_scalar_min(x0f[:], x0f[:], 62.0)
            w_all = kpool.tile([P, T * 3], F32, tag="wall")
            nc.vector.tensor_tensor(out=w_all[:], in0=ix[:], in1=x0f[:], op=ALU.subtract)
            u_all = kpool.tile([P, T * 3], F32, tag="uall")
            nc.vector.tensor_scalar(u_all[:], w_all[:], -1.0, 1.0, op0=ALU.mult, op1=ALU.add)

            x0v = x0f[:].rearrange("p (t c) -> p t c", c=3)
            bds = []  # b_d (block coord) per dim, [P, T] f32
            sds = []  # s_d (parity) per dim
            for d in range(3):
                xv = x0v[:, :, d]
                half = kpool.tile([P, T], F32, tag=f"half{d}")
                nc.vector.tensor_scalar_mul(half[:], xv, 0.5)
                hi = kpool.tile([P, T], I32, tag=f"hi{d}")
                nc.vector.tensor_copy(hi[:], half[:])
                hf = kpool.tile([P, T], F32, tag=f"hf{d}")
                nc.vector.tensor_copy(hf[:], hi[:])
                m2 = kpool.tile([P, T], F32, tag=f"m2{d}")
                nc.vector.tensor_tensor(out=m2[:], in0=hf[:], in1=half[:], op=ALU.is_gt)
                nc.vector.tensor_tensor(out=hf[:], in0=hf[:], in1=m2[:], op=ALU.subtract)
                sd = kpool.tile([P, T], F32, tag=f"sd{d}")
                nc.vector.scalar_tensor_tensor(
                    out=sd[:], in0=hf[:], scalar=-2.0, in1=xv, op0=ALU.mult, op1=ALU.add
                )
                bds.append(hf)
                sds.append(sd)
            bx, by, bz = bds
            sx, sy, sz = sds
            t1 = kpool.tile([P, T], F32, tag="t1")
            nc.vector.scalar_tensor_tensor(out=t1[:], in0=sz[:], scalar=2.0, in1=sy[:], op0=ALU.mult, op1=ALU.add)
            nc.vector.scalar_tensor_tensor(out=t1[:], in0=t1[:], scalar=2.0, in1=sx[:], op0=ALU.mult, op1=ALU.add)
            t3 = kpool.tile([P, T], F32, tag="t3")
            nc.vector.scalar_tensor_tensor(out=t3[:], in0=bz[:], scalar=32.0, in1=by[:], op0=ALU.mult, op1=ALU.add)
            nc.vector.scalar_tensor_tensor(out=t3[:], in0=t3[:], scalar=32.0, in1=bx[:], op0=ALU.mult, op1=ALU.add)
            rr = kpool.tile([P, T], F32, tag="rr")
            nc.vector.scalar_tensor_tensor(out=rr[:], in0=t1[:], scalar=float(VB), in1=t3[:], op0=ALU.mult, op1=ALU.add)
            idx_sb = kpool.tile([P, T], I32, tag="idx")
            nc.vector.tensor_copy(idx_sb[:], rr[:])

            # 8 corner-weight products W8[k][p, t], k = dz*4 + dy*2 + dx
            wv = w_all[:].rearrange("p (t c) -> p t c", c=3)
            uv = u_all[:].rearrange("p (t c) -> p t c", c=3)
            w8 = []
            for k in range(8):
                dz, dy, dx = (k >> 2) & 1, (k >> 1) & 1, k & 1
                zf = wv[:, :, 2] if dz else uv[:, :, 2]
                yf = wv[:, :, 1] if dy else uv[:, :, 1]
                xf = wv[:, :, 0] if dx else uv[:, :, 0]
                wk = kpool.tile([P, T], F32, tag=f"w8_{k}")
                nc.vector.tensor_tensor(out=wk[:], in0=zf, in1=yf, op=ALU.mult)
                nc.vector.tensor_tensor(out=wk[:], in0=wk[:], in1=xf, op=ALU.mult)
                w8.append(wk)

            # ---------- main chunk loop ----------
            for ch in range(NCH):
                # gather + interpolate + transpose 4 tiles -> c_sb [128ch, 512pts] fp16
                tr_ps = trpool.tile([P, TPC, P], F16, tag="trps")
                for tl in range(TPC):
                    t = TPC * ch + tl
                    g = gpool.tile([P, ROW], F16, tag=f"g{tl}")
                    nc.gpsimd.indirect_dma_start(
                        out=g[:],
                        out_offset=None,
                        in_=table[:],
                        in_offset=bass.IndirectOffsetOnAxis(
                            ap=idx_sb[:, t : t + 1], axis=0
                        ),
                    )
                    f32acc = fpool.tile([P, P], F32, tag=f"fa{tl}")
                    nc.vector.tensor_scalar_mul(
                        f32acc[:], g[:, 0:C], w8[0][:, t : t + 1]
                    )
                    for k in range(1, 8):
                        nc.vector.scalar_tensor_tensor(
                            out=f32acc[:],
                            in0=g[:, k * C : (k + 1) * C],
                            scalar=w8[k][:, t : t + 1],
                            in1=f32acc[:],
                            op0=ALU.mult,
                            op1=ALU.add,
                        )
                    f16t = fpool.tile([P, P], F16, tag=f"f16_{tl}")
                    nc.vector.tensor_copy(f16t[:], f32acc[:])
                    nc.tensor.transpose(tr_ps[:, tl, :], f16t[:], ident[:])
                c_sb = cpool.tile([P, NF], F16, tag="csb")
                nc.vector.tensor_copy(c_sb[:], tr_ps[:, :, :])

                # ----- MLP: residual stream lives in PSUM -----
                net = [npool.tile([P, NF], F32, tag="net", name=f"net{ch}_{m}") for m in range(2)]
                cs = slice(ch * NF, (ch + 1) * NF)
                for m in range(2):
                    ms = slice(m * P, (m + 1) * P)
                    nc.tensor.matmul(
                        net[m][:], fcp_sb[:, ms], ptp_sb[:, cs], start=True, stop=False
                    )
                for i in range(NB):
                    for m in range(2):
                        ms = slice(m * P, (m + 1) * P)
                        nc.tensor.matmul(
                            net[m][:], wc_sb[i][:, ms], c_sb[:], start=False, stop=False
                        )
                    rins = []
                    for m in range(2):
                        r = spool.tile([P, NF], F16, tag=f"rin{m}")
                        nc.scalar.activation(
                            r[:], net[m][:], AF.Relu,
                            bias=rb_sb[:, 2 * i + m : 2 * i + m + 1], scale=1.0,
                        )
                        rins.append(r)
                    hrs = []
                    for m in range(2):
                        ms = slice(m * P, (m + 1) * P)
                        hp = hpool.tile([P, NF], F32, tag="hps")
                        nc.tensor.matmul(hp[:], b0_sb[i][0][:, ms], rins[0][:], start=True, stop=False)
                        nc.tensor.matmul(hp[:], b0_sb[i][1][:, ms], rins[1][:], start=False, stop=True)
                        hr = spool.tile([P, NF], F16, tag=f"hr{m}")
                        nc.scalar.activation(
                            hr[:], hp[:], AF.Relu,
                            bias=b0b_sb[:, 2 * i + m : 2 * i + m + 1], scale=1.0,
                        )
                        hrs.append(hr)
                    last = i == NB - 1
                    for m in range(2):
                        ms = slice(m * P, (m + 1) * P)
                        nc.tensor.matmul(net[m][:], b1_sb[i][0][:, ms], hrs[0][:], start=False, stop=False)
                        nc.tensor.matmul(net[m][:], b1_sb[i][1][:, ms], hrs[1][:], start=False, stop=last)
                frs = []
                for m in range(2):
                    fr = spool.tile([P, NF], F16, tag=f"fr{m}")
                    nc.scalar.activation(
                        fr[:], net[m][:], AF.Relu,
                        bias=rb_sb[:, 10 + m : 11 + m], scale=1.0,
                    )
                    frs.append(fr)
                op_ps = opool.tile([1, NF], F32, tag="ops")
                nc.tensor.matmul(op_ps[:], ow_sb[:, 0:1], frs[0][:], start=True, stop=False)
                nc.tensor.matmul(op_ps[:], ow_sb[:, 1:2], frs[1][:], start=False, stop=True)
                nc.vector.tensor_scalar_add(out_acc[:, cs], op_ps[:], outb_sb[:1, :1])

            nc.sync.dma_start(out_dev[:], out_acc[:])

    nc.compile()
    return nc


def _build_table(grid_c):
    """grid_c: [C, 64, 64, 64] f32 (channels, z, y, x) -> [V8, ROW] fp16."""
    g = np.ascontiguousarray(np.transpose(grid_c, (1, 2, 3, 0))).astype(np.float16)
    gp = np.pad(g, ((0, 1), (0, 1), (0, 1), (0, 0)), mode="edge")  # [65,65,65,C]
    parts = []
    for sz in (0, 1):
        for sy in (0, 1):
            for sx in (0, 1):
                v = gp[sz : sz + 64, sy : sy + 64, sx : sx + 64]
                v = v.reshape(32, 2, 32, 2, 32, 2, C)
                v = np.ascontiguousarray(np.transpose(v, (0, 2, 4, 1, 3, 5, 6)))
                parts.append(v.reshape(VB, ROW))
    return np.concatenate(parts, axis=0)


def kernel(p, c_grid, fc_p_w, fc_p_b, fc_c_w, fc_c_b, b0_w, b0_b, b1_w, b1_b,
           out_w, out_b):
    p = np.asarray(p, np.float32)
    c_grid = np.asarray(c_grid, np.float32)
    fc_p_w = np.asarray(fc_p_w, np.float32)
    fc_p_b = np.asarray(fc_p_b, np.float32)
    fc_c_w = np.asarray(fc_c_w, np.float32)
    fc_c_b = np.asarray(fc_c_b, np.float32)
    b0_w = np.asarray(b0_w, np.float32)
    b0_b = np.asarray(b0_b, np.float32)
    b1_w = np.asarray(b1_w, np.float32)
    b1_b = np.asarray(b1_b, np.float32)
    out_w = np.asarray(out_w, np.float32)
    out_b = np.asarray(out_b, np.float32)

    if "nc" not in _CACHE:
        _CACHE["nc"] = _build_nc()
    nc = _CACHE["nc"]

    tables = [_build_table(c_grid[b]) for b in range(B)]

    # ---- weight prep (shared across cores) ----
    f16 = lambda a: np.ascontiguousarray(a).astype(np.float16)
    fcp = np.zeros((4, H), np.float32)
    fcp[:3] = fc_p_w.T
    fcp[3] = fc_p_b + fc_c_b[0]
    fcp = f16(fcp)
    wc = f16(np.transpose(fc_c_w, (0, 2, 1)))                       # [5,128,256]
    b0wt = f16(np.transpose(b0_w, (0, 2, 1)).reshape(NB, 2, P, H))  # K-tiles
    b1wt = f16(np.transpose(b1_w, (0, 2, 1)).reshape(NB, 2, P, H))
    oww = f16(out_w.reshape(H).reshape(2, P).T)                     # [128, 2]
    # cumulative missing-bias for relu views
    rbs = np.zeros((6, H), np.float32)
    acc = np.zeros(H, np.float32)
    for i in range(NB):
        if i > 0:
            acc = acc + fc_c_b[i]
        rbs[i] = acc
        acc = acc + b1_b[i]
    rbs[5] = acc
    rb_host = np.ascontiguousarray(
        rbs.reshape(6, 2, P).transpose(2, 0, 1).reshape(P, 12)
    ).astype(np.float32)
    b0b_host = np.ascontiguousarray(
        b0_b.reshape(NB, 2, P).transpose(2, 0, 1).reshape(P, 10)
    ).astype(np.float32)
    outb_host = np.asarray(out_b, np.float32).reshape(1, 1)

    in_maps = []
    for core in range(NCORES):
        b = core // CPB
        s = core % CPB
        sl = np.ascontiguousarray(p[b, s * NPTS : (s + 1) * NPTS])  # [NPTS, 3]
        v = sl.reshape(P, NCH, TPC, 3).transpose(3, 1, 2, 0)        # [3, 32, 4, 128]
        ptp = np.concatenate(
            [v.reshape(3, NPTS), np.ones((1, NPTS), np.float32)], axis=0
        ).astype(np.float16)
        in_maps.append(
            dict(table=tables[b], pts=sl, ptpad=np.ascontiguousarray(ptp),
                 fcp=fcp, wc=wc, b0w=b0wt, b1w=b1wt, oww=oww,
                 rb=rb_host, b0b=b0b_host, outb=outb_host)
        )

    res = run_bass_kernel_spmd(nc, in_maps, core_ids=list(range(NCORES)))

    out = np.empty((B, N, 1), np.float32)
    for core in range(NCORES):
        b = core // CPB
        s = core % CPB
        arr = res.results[core]["out_dev"][0]                       # [NPTS]
        a = arr.reshape(NCH, TPC, P).transpose(2, 0, 1).reshape(NPTS)
        out[b, s * NPTS : (s + 1) * NPTS, 0] = a
    return out


# revision 20
# speedup vs baseline: 1.3775x; 1.3775x over previous
"""Trainium2 Bass kernel for nn_LocalDecoder (ConvONet LocalDecoder: trilinear
grid sample + 5-block ResNet MLP decoder).

Strategy (8 NeuronCores):
  - Data-parallel over points: cores 0-3 take batch 0, cores 4-7 take batch 1,
    16384 points per core.
  - The feature grid is repacked on the host into an 8-shift 2x2x2-block table
    [8*32^3, 8*128] fp16: row (s, bz, by, bx) holds the 2x2x2 voxel block at
    alignment-shift s = (sz, sy, sx).  Every query point's 8 trilinear corners
    are then exactly ONE 2KB row -> one indirect-DMA descriptor per point.
  - Device computes voxel indices + trilinear weights on VectorE, gathers
    point-blocks via gpsimd indirect DMA (128 points/call), interpolates with
    fused scalar_tensor_tensor ops, transposes [pts,ch]->[ch,pts] on TensorE,
    and runs the MLP in fp16 with the residual stream resident in PSUM
    (fc_c / b1 matmuls accumulate in place; biases folded into ACT relu views).
"""

import numpy as np

import concourse.bass as bass
import concourse.bacc as bacc
import concourse.mybir as mybir
import concourse.tile as tile
from concourse.bass_utils import run_bass_kernel_spmd
from concourse.masks import make_identity

# ---- problem constants (hardcoded per contract) ----
B, N, R = 2, 65536, 64
C = 128            # grid feature channels
H = 256            # MLP hidden
NB = 5             # resnet blocks
PADDING = 0.1

NCORES = 8
CPB = NCORES // B          # cores per batch = 4
NPTS = N // CPB            # points per core = 16384
P = 128                    # partitions
T = NPTS // P              # 128 point-tiles of 128 per core
TPC = 4                    # tiles per chunk (chunk = 512 points)
NCH = T // TPC             # 32 chunks
NF = TPC * P               # chunk free dim = 512
VB = 32 * 32 * 32          # blocks per shift copy
V8 = 8 * VB                # table rows
ROW = 8 * C                # fp16 elems per table row (2KB)

SCALE = float(np.float32(63.0) / np.float32(1.0 + PADDING + 1e-3))
OFF = 31.5

F16 = mybir.dt.float16
F32 = mybir.dt.float32
I32 = mybir.dt.int32
ALU = mybir.AluOpType
AF = mybir.ActivationFunctionType

_CACHE = {}

DEFAULT_CFG = dict(rin="aa", hr="dddd", fr="ad", outadd="a", trcopy="a", gsplit=4)


def _build_nc(cfg=None):
    cfg = cfg if cfg is not None else DEFAULT_CFG
    nc = bacc.Bacc("TRN2", target_bir_lowering=False, debug=False)

    table = nc.dram_tensor("table", [V8, ROW], F16, kind="ExternalInput")
    pts = nc.dram_tensor("pts", [NPTS, 3], F32, kind="ExternalInput")
    ptpad = nc.dram_tensor("ptpad", [4, NPTS], F16, kind="ExternalInput")
    fcp = nc.dram_tensor("fcp", [4, H], F16, kind="ExternalInput")
    wc = nc.dram_tensor("wc", [NB, C, H], F16, kind="ExternalInput")
    b0w = nc.dram_tensor("b0w", [NB, 2, P, H], F16, kind="ExternalInput")
    b1w = nc.dram_tensor("b1w", [NB, 2, P, H], F16, kind="ExternalInput")
    oww = nc.dram_tensor("oww", [P, 2], F16, kind="ExternalInput")
    rb = nc.dram_tensor("rb", [P, 12], F32, kind="ExternalInput")
    b0b = nc.dram_tensor("b0b", [P, 10], F32, kind="ExternalInput")
    outb = nc.dram_tensor("outb", [1, 1], F32, kind="ExternalInput")
    out_dev = nc.dram_tensor("out_dev", [1, NPTS], F32, kind="ExternalOutput")

    with tile.TileContext(nc) as tc:
        with (
            tc.tile_pool(name="const", bufs=1) as kpool,
            tc.tile_pool(name="gather", bufs=cfg.get("gbufs", 2)) as gpool,
            tc.tile_pool(name="feat", bufs=cfg.get("fbufs", 3)) as fpool,
            tc.tile_pool(name="cs", bufs=2) as cpool,
            tc.tile_pool(name="act", bufs=cfg.get("sbufs", 2)) as spool,
            tc.tile_pool(name="pp", bufs=2) as ppool,
            tc.tile_pool(name="stage", bufs=2) as stpool,
            tc.tile_pool(name="net_ps", bufs=1, space="PSUM") as npool,
            tc.tile_pool(name="h_ps", bufs=1, space="PSUM") as hpool,
            tc.tile_pool(name="tr_ps", bufs=1, space="PSUM") as trpool,
            tc.tile_pool(name="o_ps", bufs=1, space="PSUM") as opool,
        ):
            # ---------- load constants ----------
            fcp_sb = kpool.tile([4, H], F16, tag="fcp")
            nc.sync.dma_start(fcp_sb[:], fcp[:])
            wc_sb = []
            b0_sb = []
            b1_sb = []
            for i in range(NB):
                w = kpool.tile([P, H], F16, tag=f"wc{i}")
                nc.sync.dma_start(w[:], wc[i, :, :])
                wc_sb.append(w)
                r0 = []
                r1 = []
                for kk in range(2):
                    a = kpool.tile([P, H], F16, tag=f"b0_{i}_{kk}")
                    nc.sync.dma_start(a[:], b0w[i, kk, :, :])
                    r0.append(a)
                    b = kpool.tile([P, H], F16, tag=f"b1_{i}_{kk}")
                    nc.sync.dma_start(b[:], b1w[i, kk, :, :])
                    r1.append(b)
                b0_sb.append(r0)
                b1_sb.append(r1)
            ow_sb = kpool.tile([P, 2], F16, tag="oww")
            nc.sync.dma_start(ow_sb[:], oww[:])
            rb_sb = kpool.tile([P, 12], F32, tag="rb")
            nc.sync.dma_start(rb_sb[:], rb[:])
            b0b_sb = kpool.tile([P, 10], F32, tag="b0b")
            nc.sync.dma_start(b0b_sb[:], b0b[:])
            outb_sb = kpool.tile([1, 1], F32, tag="outb")
            nc.sync.dma_start(outb_sb[:], outb[:])
            ident = kpool.tile([P, P], F16, tag="ident")
            make_identity(nc, ident[:])

            # ---------- index / weight precompute (VectorE) ----------
            # p_slab[p, t*3+c] = pts[p*T + t, c].  Emitted in two column
            # ranges (first pair's tiles, then the rest) so the pipeline can
            # start gathering immediately.
            p_slab = kpool.tile([P, T * 3], F32, tag="pslab")
            nc.sync.dma_start(
                p_slab[:], pts[:].rearrange("(p t) c -> p (t c)", p=P)
            )
            ix = kpool.tile([P, T * 3], F32, tag="ix")
            x0i = kpool.tile([P, T * 3], I32, tag="x0i")
            x0f = kpool.tile([P, T * 3], F32, tag="x0f")
            fixm = kpool.tile([P, T * 3], F32, tag="fixm")
            w_all = kpool.tile([P, T * 3], F32, tag="wall")
            u_all = kpool.tile([P, T * 3], F32, tag="uall")
            dimt = {}
            for d in range(3):
                for nm in ("half", "hi", "hf", "m2", "sd"):
                    dt_ = I32 if nm == "hi" else F32
                    dimt[(nm, d)] = kpool.tile([P, T], dt_, tag=f"{nm}{d}", name=f"{nm}{d}")
            t1 = kpool.tile([P, T], F32, tag="t1")
            t3 = kpool.tile([P, T], F32, tag="t3")
            rr = kpool.tile([P, T], F32, tag="rr")
            idx_sb = kpool.tile([P, T], I32, tag="idx")
            w8 = [kpool.tile([P, T], F32, tag=f"w8_{k}", name=f"w8_{k}") for k in range(8)]

            def emit_idx(lo, hi):
                s3 = slice(lo * 3, hi * 3)
                s1 = slice(lo, hi)
                nc.vector.tensor_scalar(ix[:, s3], p_slab[:, s3], SCALE, OFF, op0=ALU.mult, op1=ALU.add)
                nc.vector.tensor_scalar_max(ix[:, s3], ix[:, s3], 0.0)
                nc.vector.tensor_scalar_min(ix[:, s3], ix[:, s3], 63.0)
                nc.vector.tensor_copy(x0i[:, s3], ix[:, s3])
                nc.vector.tensor_copy(x0f[:, s3], x0i[:, s3])
                nc.vector.tensor_tensor(out=fixm[:, s3], in0=x0f[:, s3], in1=ix[:, s3], op=ALU.is_gt)
                nc.vector.tensor_tensor(out=x0f[:, s3], in0=x0f[:, s3], in1=fixm[:, s3], op=ALU.subtract)
                nc.vector.tensor_scalar_min(x0f[:, s3], x0f[:, s3], 62.0)
                nc.vector.tensor_tensor(out=w_all[:, s3], in0=ix[:, s3], in1=x0f[:, s3], op=ALU.subtract)
                nc.vector.tensor_scalar(u_all[:, s3], w_all[:, s3], -1.0, 1.0, op0=ALU.mult, op1=ALU.add)

                x0v = x0f[:].rearrange("p (t c) -> p t c", c=3)
                bds = []
                sds = []
                for d in range(3):
                    xv = x0v[:, s1, d]
                    half, hi_, hf, m2, sd = (dimt[(nm, d)] for nm in ("half", "hi", "hf", "m2", "sd"))
                    nc.vector.tensor_scalar_mul(half[:, s1], xv, 0.5)
                    nc.vector.tensor_copy(hi_[:, s1], half[:, s1])
                    nc.vector.tensor_copy(hf[:, s1], hi_[:, s1])
                    nc.vector.tensor_tensor(out=m2[:, s1], in0=hf[:, s1], in1=half[:, s1], op=ALU.is_gt)
                    nc.vector.tensor_tensor(out=hf[:, s1], in0=hf[:, s1], in1=m2[:, s1], op=ALU.subtract)
                    nc.vector.scalar_tensor_tensor(
                        out=sd[:, s1], in0=hf[:, s1], scalar=-2.0, in1=xv, op0=ALU.mult, op1=ALU.add
                    )
                    bds.append(hf)
                    sds.append(sd)
                bx, by, bz = bds
                sx, sy, sz = sds
                nc.vector.scalar_tensor_tensor(out=t1[:, s1], in0=sz[:, s1], scalar=2.0, in1=sy[:, s1], op0=ALU.mult, op1=ALU.add)
                nc.vector.scalar_tensor_tensor(out=t1[:, s1], in0=t1[:, s1], scalar=2.0, in1=sx[:, s1], op0=ALU.mult, op1=ALU.add)
                nc.vector.scalar_tensor_tensor(out=t3[:, s1], in0=bz[:, s1], scalar=32.0, in1=by[:, s1], op0=ALU.mult, op1=ALU.add)
                nc.vector.scalar_tensor_tensor(out=t3[:, s1], in0=t3[:, s1], scalar=32.0, in1=bx[:, s1], op0=ALU.mult, op1=ALU.add)
                nc.vector.scalar_tensor_tensor(out=rr[:, s1], in0=t1[:, s1], scalar=float(VB), in1=t3[:, s1], op0=ALU.mult, op1=ALU.add)
                nc.vector.tensor_copy(idx_sb[:, s1], rr[:, s1])

                # 8 corner-weight products W8[k][p, t], k = dz*4 + dy*2 + dx
                wv = w_all[:].rearrange("p (t c) -> p t c", c=3)
                uv = u_all[:].rearrange("p (t c) -> p t c", c=3)
                for k in range(8):
                    dz, dy, dx = (k >> 2) & 1, (k >> 1) & 1, k & 1
                    zf = (wv if dz else uv)[:, s1, 2]
                    yf = (wv if dy else uv)[:, s1, 1]
                    xf = (wv if dx else uv)[:, s1, 0]
                    wk = w8[k]
                    nc.vector.tensor_tensor(out=wk[:, s1], in0=zf, in1=yf, op=ALU.mult)
                    nc.vector.tensor_tensor(out=wk[:, s1], in0=wk[:, s1], in1=xf, op=ALU.mult)

            emit_idx(0, 2 * TPC)
            emit_idx(2 * TPC, T)

            # ---------- main loop: chunk PAIRS, MLPs interleaved ----------
            # Two independent per-chunk dependency chains fill each other's
            # engine stalls; relu engine alternates by chunk parity so the
            # two chains mostly use disjoint engines (ACT vs DVE).
            def relu_op(on_dve, dst, src, bias_ap, split=False):
                if split:
                    hf = NF // 2
                    nc.scalar.activation(
                        dst[:, :hf], src[:, :hf], AF.Relu, bias=bias_ap, scale=1.0
                    )
                    nc.vector.tensor_scalar(
                        dst[:, hf:], src[:, hf:], bias_ap, 0.0, op0=ALU.add, op1=ALU.max
                    )
                elif not on_dve:
                    nc.scalar.activation(dst[:], src[:], AF.Relu, bias=bias_ap, scale=1.0)
                else:
                    nc.vector.tensor_scalar(
                        dst[:], src[:], bias_ap, 0.0, op0=ALU.add, op1=ALU.max
                    )

            for pc in range(NCH // 2):
                chunks = (2 * pc, 2 * pc + 1)
                ptp_pair = ppool.tile([4, 2 * NF], F16, tag="ptpp", name=f"ptpp{pc}")
                nc.sync.dma_start(
                    ptp_pair[:], ptpad[:, 2 * pc * NF : 2 * (pc + 1) * NF]
                )
                stage = stpool.tile([1, 2 * NF], F32, tag="stage", name=f"stage{pc}")
                csbs = []
                tr_ps = trpool.tile([P, 2 * TPC, P], F16, tag="trps", name=f"trps{pc}")
                gts = {}
                for ci, ch in enumerate(chunks):
                    for tl in range(TPC):
                        t = TPC * ch + tl
                        g = gpool.tile([P, ROW], F16, tag=f"g{ci}_{tl}", name=f"g{ch}_{tl}")
                        nc.gpsimd.indirect_dma_start(
                            out=g[:],
                            out_offset=None,
                            in_=table[:],
                            in_offset=bass.IndirectOffsetOnAxis(
                                ap=idx_sb[:, t : t + 1], axis=0
                            ),
                        )
                        gts[(ci, tl)] = g
                gsplit = cfg.get("gsplit", 2)
                for ci, ch in enumerate(chunks):
                    for tl in range(TPC):
                        t = TPC * ch + tl
                        g = gts[(ci, tl)]
                        eng = nc.vector if tl < gsplit else nc.gpsimd
                        facc = fpool.tile([P, P], F16, tag=f"fa{ci}_{tl}", name=f"fa{ch}_{tl}")
                        eng.tensor_scalar_mul(
                            facc[:], g[:, 0:C], w8[0][:, t : t + 1]
                        )
                        for k in range(1, 8):
                            eng.scalar_tensor_tensor(
                                out=facc[:],
                                in0=g[:, k * C : (k + 1) * C],
                                scalar=w8[k][:, t : t + 1],
                                in1=facc[:],
                                op0=ALU.mult,
                                op1=ALU.add,
                            )
                        nc.tensor.transpose(tr_ps[:, ci * TPC + tl, :], facc[:], ident[:])
                    c_sb = cpool.tile([P, NF], F16, tag=f"csb{ci}", name=f"csb{ch}")
                    if cfg.get('trcopy', 'v') == 'a':
                        nc.scalar.copy(c_sb[:], tr_ps[:, ci * TPC : (ci + 1) * TPC, :])
                    else:
                        nc.vector.tensor_copy(c_sb[:], tr_ps[:, ci * TPC : (ci + 1) * TPC, :])
                    csbs.append(c_sb)

                # ----- interleaved MLPs: residual streams live in PSUM -----
                nets = []
                for ci, ch in enumerate(chunks):
                    net = [
                        npool.tile([P, NF], F32, tag=f"net{ci}_{m}", name=f"net{ch}_{m}")
                        for m in range(2)
                    ]
                    for m in range(2):
                        ms = slice(m * P, (m + 1) * P)
                        nc.tensor.matmul(
                            net[m][:], fcp_sb[:, ms],
                            ptp_pair[:, ci * NF : (ci + 1) * NF],
                            start=True, stop=False,
                        )
                    nets.append(net)
                for i in range(NB):
                    rins2 = []
                    for ci, ch in enumerate(chunks):
                        net = nets[ci]
                        for m in range(2):
                            ms = slice(m * P, (m + 1) * P)
                            nc.tensor.matmul(
                                net[m][:], wc_sb[i][:, ms], csbs[ci][:], start=False, stop=False
                            )
                        rins = []
                        for m in range(2):
                            r = spool.tile([P, NF], F16, tag=f"rin{ci}_{m}", name=f"rin{ch}_{m}")
                            relu_op(cfg.get('rin', 'aa')[ci] == 'd', r, net[m], rb_sb[:, 2 * i + m : 2 * i + m + 1], split=cfg.get('split_rin', False))
                            rins.append(r)
                        rins2.append(rins)
                    hrs2 = []
                    for ci, ch in enumerate(chunks):
                        hrs = []
                        for m in range(2):
                            ms = slice(m * P, (m + 1) * P)
                            hp = hpool.tile([P, NF], F32, tag=f"hps{m}", name=f"hps{ch}_{m}")
                            nc.tensor.matmul(hp[:], b0_sb[i][0][:, ms], rins2[ci][0][:], start=True, stop=False)
                            nc.tensor.matmul(hp[:], b0_sb[i][1][:, ms], rins2[ci][1][:], start=False, stop=True)
                            hr = spool.tile([P, NF], F16, tag=f"hr{ci}_{m}", name=f"hr{ch}_{m}")
                            relu_op(cfg.get('hr', 'adad')[2 * ci + m] == 'd', hr, hp, b0b_sb[:, 2 * i + m : 2 * i + m + 1], split=cfg.get('split_hr', False))
                            hrs.append(hr)
                        hrs2.append(hrs)
                    last = i == NB - 1
                    for ci, ch in enumerate(chunks):
                        net = nets[ci]
                        for m in range(2):
                            ms = slice(m * P, (m + 1) * P)
                            nc.tensor.matmul(net[m][:], b1_sb[i][0][:, ms], hrs2[ci][0][:], start=False, stop=False)
                            nc.tensor.matmul(net[m][:], b1_sb[i][1][:, ms], hrs2[ci][1][:], start=False, stop=last)
                for ci, ch in enumerate(chunks):
                    net = nets[ci]
                    frs = []
                    for m in range(2):
                        fr = spool.tile([P, NF], F16, tag=f"fr{ci}_{m}", name=f"fr{ch}_{m}")
                        relu_op(cfg.get('fr', 'dd')[ci] == 'd', fr, net[m], rb_sb[:, 10 + m : 11 + m])
                        frs.append(fr)
                    op_ps = opool.tile([1, NF], F32, tag="ops", name=f"ops{ch}")
                    nc.tensor.matmul(op_ps[:], ow_sb[:, 0:1], frs[0][:], start=True, stop=False)
                    nc.tensor.matmul(op_ps[:], ow_sb[:, 1:2], frs[1][:], start=False, stop=True)
                    if cfg.get('outadd', 'v') == 'a':
                        nc.scalar.activation(
                            stage[:, ci * NF : (ci + 1) * NF], op_ps[:],
                            AF.Identity, bias=outb_sb[:1, :1], scale=1.0,
                        )
                    else:
                        nc.vector.tensor_scalar_add(
                            stage[:, ci * NF : (ci + 1) * NF], op_ps[:], outb_sb[:1, :1]
                        )
                nc.sync.dma_start(
                    out_dev[:, 2 * pc * NF : 2 * (pc + 1) * NF], stage[:]
                )

    nc.compile()
    return nc


def _build_table(grid_c):
    """grid_c: [C, 64, 64, 64] f32 (channels, z, y, x) -> [V8, ROW] fp16."""
    g = np.ascontiguousarray(np.transpose(grid_c, (1, 2, 3, 0))).astype(np.float16)
    gp = np.pad(g, ((0, 1), (0, 1), (0, 1), (0, 0)), mode="edge")  # [65,65,65,C]
    parts = []
    for sz in (0, 1):
        for sy in (0, 1):
            for sx in (0, 1):
                v = gp[sz : sz + 64, sy : sy + 64, sx : sx + 64]
                v = v.reshape(32, 2, 32, 2, 32, 2, C)
                v = np.ascontiguousarray(np.transpose(v, (0, 2, 4, 1, 3, 5, 6)))
                parts.append(v.reshape(VB, ROW))
    return np.concatenate(parts, axis=0)


def kernel(p, c_grid, fc_p_w, fc_p_b, fc_c_w, fc_c_b, b0_w, b0_b, b1_w, b1_b,
           out_w, out_b):
    p = np.asarray(p, np.float32)
    c_grid = np.asarray(c_grid, np.float32)
    fc_p_w = np.asarray(fc_p_w, np.float32)
    fc_p_b = np.asarray(fc_p_b, np.float32)
    fc_c_w = np.asarray(fc_c_w, np.float32)
    fc_c_b = np.asarray(fc_c_b, np.float32)
    b0_w = np.asarray(b0_w, np.float32)
    b0_b = np.asarray(b0_b, np.float32)
    b1_w = np.asarray(b1_w, np.float32)
    b1_b = np.asarray(b1_b, np.float32)
    out_w = np.asarray(out_w, np.float32)
    out_b = np.asarray(out_b, np.float32)

    if "nc" not in _CACHE:
        _CACHE["nc"] = _build_nc()
    nc = _CACHE["nc"]

    tables = [_build_table(c_grid[b]) for b in range(B)]

    # ---- weight prep (shared across cores) ----
    f16 = lambda a: np.ascontiguousarray(a).astype(np.float16)
    fcp = np.zeros((4, H), np.float32)
    fcp[:3] = fc_p_w.T
    fcp[3] = fc_p_b + fc_c_b[0]
    fcp = f16(fcp)
    wc = f16(np.transpose(fc_c_w, (0, 2, 1)))                       # [5,128,256]
    b0wt = f16(np.transpose(b0_w, (0, 2, 1)).reshape(NB, 2, P, H))  # K-tiles
    b1wt = f16(np.transpose(b1_w, (0, 2, 1)).reshape(NB, 2, P, H))
    oww = f16(out_w.reshape(H).reshape(2, P).T)                     # [128, 2]
    # cumulative missing-bias for relu views
    rbs = np.zeros((6, H), np.float32)
    acc = np.zeros(H, np.float32)
    for i in range(NB):
        if i > 0:
            acc = acc + fc_c_b[i]
        rbs[i] = acc
        acc = acc + b1_b[i]
    rbs[5] = acc
    rb_host = np.ascontiguousarray(
        rbs.reshape(6, 2, P).transpose(2, 0, 1).reshape(P, 12)
    ).astype(np.float32)
    b0b_host = np.ascontiguousarray(
        b0_b.reshape(NB, 2, P).transpose(2, 0, 1).reshape(P, 10)
    ).astype(np.float32)
    outb_host = np.asarray(out_b, np.float32).reshape(1, 1)

    in_maps = []
    for core in range(NCORES):
        b = core // CPB
        s = core % CPB
        sl = np.ascontiguousarray(p[b, s * NPTS : (s + 1) * NPTS])  # [NPTS, 3]
        v = sl.reshape(P, NCH, TPC, 3).transpose(3, 1, 2, 0)        # [3, 32, 4, 128]
        ptp = np.concatenate(
            [v.reshape(3, NPTS), np.ones((1, NPTS), np.float32)], axis=0
        ).astype(np.float16)
        in_maps.append(
            dict(table=tables[b], pts=sl, ptpad=np.ascontiguousarray(ptp),
                 fcp=fcp, wc=wc, b0w=b0wt, b1w=b1wt, oww=oww,
                 rb=rb_host, b0b=b0b_host, outb=outb_host)
        )

    res = run_bass_kernel_spmd(nc, in_maps, core_ids=list(range(NCORES)))

    out = np.empty((B, N, 1), np.float32)
    for core in range(NCORES):
        b = core // CPB
        s = core % CPB
        arr = res.results[core]["out_dev"][0]                       # [NPTS]
        a = arr.reshape(NCH, TPC, P).transpose(2, 0, 1).reshape(NPTS)
        out[b, s * NPTS : (s + 1) * NPTS, 0] = a
    return out


# revision 23
# speedup vs baseline: 1.3878x; 1.0074x over previous
"""Trainium2 Bass kernel for nn_LocalDecoder (ConvONet LocalDecoder: trilinear
grid sample + 5-block ResNet MLP decoder).

Strategy (8 NeuronCores):
  - Data-parallel over points: cores 0-3 take batch 0, cores 4-7 take batch 1,
    16384 points per core.
  - The feature grid is repacked on the host into an 8-shift 2x2x2-block table
    [8*32^3, 8*128] fp16: row (s, bz, by, bx) holds the 2x2x2 voxel block at
    alignment-shift s = (sz, sy, sx).  Every query point's 8 trilinear corners
    are then exactly ONE 2KB row -> one indirect-DMA descriptor per point.
  - Device computes voxel indices + trilinear weights on VectorE, gathers
    point-blocks via gpsimd indirect DMA (128 points/call), interpolates with
    fused scalar_tensor_tensor ops, transposes [pts,ch]->[ch,pts] on TensorE,
    and runs the MLP in fp16 with the residual stream resident in PSUM
    (fc_c / b1 matmuls accumulate in place; biases folded into ACT relu views).
"""

import numpy as np

import concourse.bass as bass
import concourse.bacc as bacc
import concourse.mybir as mybir
import concourse.tile as tile
from concourse.bass_utils import run_bass_kernel_spmd
from concourse.masks import make_identity

# ---- problem constants (hardcoded per contract) ----
B, N, R = 2, 65536, 64
C = 128            # grid feature channels
H = 256            # MLP hidden
NB = 5             # resnet blocks
PADDING = 0.1

NCORES = 8
CPB = NCORES // B          # cores per batch = 4
NPTS = N // CPB            # points per core = 16384
P = 128                    # partitions
T = NPTS // P              # 128 point-tiles of 128 per core
TPC = 4                    # tiles per chunk (chunk = 512 points)
NCH = T // TPC             # 32 chunks
NF = TPC * P               # chunk free dim = 512
VB = 32 * 32 * 32          # blocks per shift copy
V8 = 8 * VB                # table rows
ROW = 8 * C                # fp16 elems per table row (2KB)

SCALE = float(np.float32(63.0) / np.float32(1.0 + PADDING + 1e-3))
OFF = 31.5

F16 = mybir.dt.float16
F32 = mybir.dt.float32
I32 = mybir.dt.int32
ALU = mybir.AluOpType
AF = mybir.ActivationFunctionType

_CACHE = {}

DEFAULT_CFG = dict(rin="aa", hr="dddd", fr="ad", outadd="a", trcopy="a", gsplit=4,
                   cbufs=3, gbufs=3, sbufs=3)


def _build_nc(cfg=None):
    cfg = cfg if cfg is not None else DEFAULT_CFG
    nc = bacc.Bacc("TRN2", target_bir_lowering=False, debug=False)

    table = nc.dram_tensor("table", [V8, ROW], F16, kind="ExternalInput")
    pts = nc.dram_tensor("pts", [NPTS, 3], F32, kind="ExternalInput")
    ptpad = nc.dram_tensor("ptpad", [4, NPTS], F16, kind="ExternalInput")
    fcp = nc.dram_tensor("fcp", [4, H], F16, kind="ExternalInput")
    wc = nc.dram_tensor("wc", [NB, C, H], F16, kind="ExternalInput")
    b0w = nc.dram_tensor("b0w", [NB, 2, P, H], F16, kind="ExternalInput")
    b1w = nc.dram_tensor("b1w", [NB, 2, P, H], F16, kind="ExternalInput")
    oww = nc.dram_tensor("oww", [P, 2], F16, kind="ExternalInput")
    rb = nc.dram_tensor("rb", [P, 12], F32, kind="ExternalInput")
    b0b = nc.dram_tensor("b0b", [P, 10], F32, kind="ExternalInput")
    outb = nc.dram_tensor("outb", [1, 1], F32, kind="ExternalInput")
    out_dev = nc.dram_tensor("out_dev", [1, NPTS], F32, kind="ExternalOutput")

    with tile.TileContext(nc) as tc:
        with (
            tc.tile_pool(name="const", bufs=1) as kpool,
            tc.tile_pool(name="gather", bufs=cfg.get("gbufs", 2)) as gpool,
            tc.tile_pool(name="feat", bufs=cfg.get("fbufs", 3)) as fpool,
            tc.tile_pool(name="cs", bufs=cfg.get("cbufs", 2)) as cpool,
            tc.tile_pool(name="act", bufs=cfg.get("sbufs", 2)) as spool,
            tc.tile_pool(name="pp", bufs=2) as ppool,
            tc.tile_pool(name="stage", bufs=2) as stpool,
            tc.tile_pool(name="net_ps", bufs=1, space="PSUM") as npool,
            tc.tile_pool(name="h_ps", bufs=1, space="PSUM") as hpool,
            tc.tile_pool(name="tr_ps", bufs=1, space="PSUM") as trpool,
            tc.tile_pool(name="o_ps", bufs=1, space="PSUM") as opool,
        ):
            # ---------- load constants ----------
            fcp_sb = kpool.tile([4, H], F16, tag="fcp")
            nc.sync.dma_start(fcp_sb[:], fcp[:])
            wc_sb = []
            b0_sb = []
            b1_sb = []
            for i in range(NB):
                w = kpool.tile([P, H], F16, tag=f"wc{i}")
                nc.sync.dma_start(w[:], wc[i, :, :])
                wc_sb.append(w)
                r0 = []
                r1 = []
                for kk in range(2):
                    a = kpool.tile([P, H], F16, tag=f"b0_{i}_{kk}")
                    nc.sync.dma_start(a[:], b0w[i, kk, :, :])
                    r0.append(a)
                    b = kpool.tile([P, H], F16, tag=f"b1_{i}_{kk}")
                    nc.sync.dma_start(b[:], b1w[i, kk, :, :])
                    r1.append(b)
                b0_sb.append(r0)
                b1_sb.append(r1)
            ow_sb = kpool.tile([P, 2], F16, tag="oww")
            nc.sync.dma_start(ow_sb[:], oww[:])
            rb_sb = kpool.tile([P, 12], F32, tag="rb")
            nc.sync.dma_start(rb_sb[:], rb[:])
            b0b_sb = kpool.tile([P, 10], F32, tag="b0b")
            nc.sync.dma_start(b0b_sb[:], b0b[:])
            outb_sb = kpool.tile([1, 1], F32, tag="outb")
            nc.sync.dma_start(outb_sb[:], outb[:])
            ident = kpool.tile([P, P], F16, tag="ident")
            make_identity(nc, ident[:])

            # ---------- index / weight precompute (VectorE) ----------
            # p_slab[p, t*3+c] = pts[p*T + t, c].  Emitted in two column
            # ranges (first pair's tiles, then the rest) so the pipeline can
            # start gathering immediately.
            p_slab = kpool.tile([P, T * 3], F32, tag="pslab")
            nc.sync.dma_start(
                p_slab[:], pts[:].rearrange("(p t) c -> p (t c)", p=P)
            )
            ix = kpool.tile([P, T * 3], F32, tag="ix")
            x0i = kpool.tile([P, T * 3], I32, tag="x0i")
            x0f = kpool.tile([P, T * 3], F32, tag="x0f")
            fixm = kpool.tile([P, T * 3], F32, tag="fixm")
            w_all = kpool.tile([P, T * 3], F32, tag="wall")
            u_all = kpool.tile([P, T * 3], F32, tag="uall")
            dimt = {}
            for d in range(3):
                for nm in ("half", "hi", "hf", "m2", "sd"):
                    dt_ = I32 if nm == "hi" else F32
                    dimt[(nm, d)] = kpool.tile([P, T], dt_, tag=f"{nm}{d}", name=f"{nm}{d}")
            t1 = kpool.tile([P, T], F32, tag="t1")
            t3 = kpool.tile([P, T], F32, tag="t3")
            rr = kpool.tile([P, T], F32, tag="rr")
            idx_sb = kpool.tile([P, T], I32, tag="idx")
            w8 = [kpool.tile([P, T], F32, tag=f"w8_{k}", name=f"w8_{k}") for k in range(8)]

            def emit_idx(lo, hi):
                s3 = slice(lo * 3, hi * 3)
                s1 = slice(lo, hi)
                nc.vector.tensor_scalar(ix[:, s3], p_slab[:, s3], SCALE, OFF, op0=ALU.mult, op1=ALU.add)
                nc.vector.tensor_scalar_max(ix[:, s3], ix[:, s3], 0.0)
                nc.vector.tensor_scalar_min(ix[:, s3], ix[:, s3], 63.0)
                nc.vector.tensor_copy(x0i[:, s3], ix[:, s3])
                nc.vector.tensor_copy(x0f[:, s3], x0i[:, s3])
                nc.vector.tensor_tensor(out=fixm[:, s3], in0=x0f[:, s3], in1=ix[:, s3], op=ALU.is_gt)
                nc.vector.tensor_tensor(out=x0f[:, s3], in0=x0f[:, s3], in1=fixm[:, s3], op=ALU.subtract)
                nc.vector.tensor_scalar_min(x0f[:, s3], x0f[:, s3], 62.0)
                nc.vector.tensor_tensor(out=w_all[:, s3], in0=ix[:, s3], in1=x0f[:, s3], op=ALU.subtract)
                nc.vector.tensor_scalar(u_all[:, s3], w_all[:, s3], -1.0, 1.0, op0=ALU.mult, op1=ALU.add)

                x0v = x0f[:].rearrange("p (t c) -> p t c", c=3)
                bds = []
                sds = []
                for d in range(3):
                    xv = x0v[:, s1, d]
                    half, hi_, hf, m2, sd = (dimt[(nm, d)] for nm in ("half", "hi", "hf", "m2", "sd"))
                    nc.vector.tensor_scalar_mul(half[:, s1], xv, 0.5)
                    nc.vector.tensor_copy(hi_[:, s1], half[:, s1])
                    nc.vector.tensor_copy(hf[:, s1], hi_[:, s1])
                    nc.vector.tensor_tensor(out=m2[:, s1], in0=hf[:, s1], in1=half[:, s1], op=ALU.is_gt)
                    nc.vector.tensor_tensor(out=hf[:, s1], in0=hf[:, s1], in1=m2[:, s1], op=ALU.subtract)
                    nc.vector.scalar_tensor_tensor(
                        out=sd[:, s1], in0=hf[:, s1], scalar=-2.0, in1=xv, op0=ALU.mult, op1=ALU.add
                    )
                    bds.append(hf)
                    sds.append(sd)
                bx, by, bz = bds
                sx, sy, sz = sds
                nc.vector.scalar_tensor_tensor(out=t1[:, s1], in0=sz[:, s1], scalar=2.0, in1=sy[:, s1], op0=ALU.mult, op1=ALU.add)
                nc.vector.scalar_tensor_tensor(out=t1[:, s1], in0=t1[:, s1], scalar=2.0, in1=sx[:, s1], op0=ALU.mult, op1=ALU.add)
                nc.vector.scalar_tensor_tensor(out=t3[:, s1], in0=bz[:, s1], scalar=32.0, in1=by[:, s1], op0=ALU.mult, op1=ALU.add)
                nc.vector.scalar_tensor_tensor(out=t3[:, s1], in0=t3[:, s1], scalar=32.0, in1=bx[:, s1], op0=ALU.mult, op1=ALU.add)
                nc.vector.scalar_tensor_tensor(out=rr[:, s1], in0=t1[:, s1], scalar=float(VB), in1=t3[:, s1], op0=ALU.mult, op1=ALU.add)
                nc.vector.tensor_copy(idx_sb[:, s1], rr[:, s1])

                # 8 corner-weight products W8[k][p, t], k = dz*4 + dy*2 + dx
                wv = w_all[:].rearrange("p (t c) -> p t c", c=3)
                uv = u_all[:].rearrange("p (t c) -> p t c", c=3)
                for k in range(8):
                    dz, dy, dx = (k >> 2) & 1, (k >> 1) & 1, k & 1
                    zf = (wv if dz else uv)[:, s1, 2]
                    yf = (wv if dy else uv)[:, s1, 1]
                    xf = (wv if dx else uv)[:, s1, 0]
                    wk = w8[k]
                    nc.vector.tensor_tensor(out=wk[:, s1], in0=zf, in1=yf, op=ALU.mult)
                    nc.vector.tensor_tensor(out=wk[:, s1], in0=wk[:, s1], in1=xf, op=ALU.mult)

            emit_idx(0, 2 * TPC)
            emit_idx(2 * TPC, T)

            # ---------- main loop: chunk PAIRS, MLPs interleaved ----------
            # Two independent per-chunk dependency chains fill each other's
            # engine stalls; relu engine alternates by chunk parity so the
            # two chains mostly use disjoint engines (ACT vs DVE).
            def relu_op(on_dve, dst, src, bias_ap, split=False):
                if split:
                    hf = NF // 2
                    nc.scalar.activation(
                        dst[:, :hf], src[:, :hf], AF.Relu, bias=bias_ap, scale=1.0
                    )
                    nc.vector.tensor_scalar(
                        dst[:, hf:], src[:, hf:], bias_ap, 0.0, op0=ALU.add, op1=ALU.max
                    )
                elif not on_dve:
                    nc.scalar.activation(dst[:], src[:], AF.Relu, bias=bias_ap, scale=1.0)
                else:
                    nc.vector.tensor_scalar(
                        dst[:], src[:], bias_ap, 0.0, op0=ALU.add, op1=ALU.max
                    )

            for pc in range(NCH // 2):
                chunks = (2 * pc, 2 * pc + 1)
                ptp_pair = ppool.tile([4, 2 * NF], F16, tag="ptpp", name=f"ptpp{pc}")
                nc.sync.dma_start(
                    ptp_pair[:], ptpad[:, 2 * pc * NF : 2 * (pc + 1) * NF]
                )
                stage = stpool.tile([1, 2 * NF], F32, tag="stage", name=f"stage{pc}")
                csbs = []
                tr_ps = trpool.tile([P, 2 * TPC, P], F16, tag="trps", name=f"trps{pc}")
                gts = {}
                for ci, ch in enumerate(chunks):
                    for tl in range(TPC):
                        t = TPC * ch + tl
                        g = gpool.tile([P, ROW], F16, tag=f"g{ci}_{tl}", name=f"g{ch}_{tl}")
                        nc.gpsimd.indirect_dma_start(
                            out=g[:],
                            out_offset=None,
                            in_=table[:],
                            in_offset=bass.IndirectOffsetOnAxis(
                                ap=idx_sb[:, t : t + 1], axis=0
                            ),
                        )
                        gts[(ci, tl)] = g
                gsplit = cfg.get("gsplit", 2)
                for ci, ch in enumerate(chunks):
                    for tl in range(TPC):
                        t = TPC * ch + tl
                        g = gts[(ci, tl)]
                        eng = nc.vector if tl < gsplit else nc.gpsimd
                        facc = fpool.tile([P, P], F16, tag=f"fa{ci}_{tl}", name=f"fa{ch}_{tl}")
                        eng.tensor_scalar_mul(
                            facc[:], g[:, 0:C], w8[0][:, t : t + 1]
                        )
                        for k in range(1, 8):
                            eng.scalar_tensor_tensor(
                                out=facc[:],
                                in0=g[:, k * C : (k + 1) * C],
                                scalar=w8[k][:, t : t + 1],
                                in1=facc[:],
                                op0=ALU.mult,
                                op1=ALU.add,
                            )
                        nc.tensor.transpose(tr_ps[:, ci * TPC + tl, :], facc[:], ident[:])
                    c_sb = cpool.tile([P, NF], F16, tag=f"csb{ci}", name=f"csb{ch}")
                    if cfg.get('trcopy', 'v') == 'a':
                        nc.scalar.copy(c_sb[:], tr_ps[:, ci * TPC : (ci + 1) * TPC, :])
                    else:
                        nc.vector.tensor_copy(c_sb[:], tr_ps[:, ci * TPC : (ci + 1) * TPC, :])
                    csbs.append(c_sb)

                # ----- interleaved MLPs: residual streams live in PSUM -----
                nets = []
                for ci, ch in enumerate(chunks):
                    net = [
                        npool.tile([P, NF], F32, tag=f"net{ci}_{m}", name=f"net{ch}_{m}")
                        for m in range(2)
                    ]
                    for m in range(2):
                        ms = slice(m * P, (m + 1) * P)
                        nc.tensor.matmul(
                            net[m][:], fcp_sb[:, ms],
                            ptp_pair[:, ci * NF : (ci + 1) * NF],
                            start=True, stop=False,
                        )
                    nets.append(net)
                for i in range(NB):
                    rins2 = []
                    for ci, ch in enumerate(chunks):
                        net = nets[ci]
                        for m in range(2):
                            ms = slice(m * P, (m + 1) * P)
                            nc.tensor.matmul(
                                net[m][:], wc_sb[i][:, ms], csbs[ci][:], start=False, stop=False
                            )
                        rins = []
                        for m in range(2):
                            r = spool.tile([P, NF], F16, tag=f"rin{ci}_{m}", name=f"rin{ch}_{m}")
                            relu_op(cfg.get('rin', 'aa')[ci] == 'd', r, net[m], rb_sb[:, 2 * i + m : 2 * i + m + 1], split=cfg.get('split_rin', False))
                            rins.append(r)
                        rins2.append(rins)
                    hrs2 = []
                    for ci, ch in enumerate(chunks):
                        hrs = []
                        for m in range(2):
                            ms = slice(m * P, (m + 1) * P)
                            hp = hpool.tile([P, NF], F32, tag=f"hps{m}", name=f"hps{ch}_{m}", bufs=(2 if m == 0 else 1))
                            nc.tensor.matmul(hp[:], b0_sb[i][0][:, ms], rins2[ci][0][:], start=True, stop=False)
                            nc.tensor.matmul(hp[:], b0_sb[i][1][:, ms], rins2[ci][1][:], start=False, stop=True)
                            hr = spool.tile([P, NF], F16, tag=f"hr{ci}_{m}", name=f"hr{ch}_{m}")
                            relu_op(cfg.get('hr', 'adad')[2 * ci + m] == 'd', hr, hp, b0b_sb[:, 2 * i + m : 2 * i + m + 1], split=cfg.get('split_hr', False))
                            hrs.append(hr)
                        hrs2.append(hrs)
                    last = i == NB - 1
                    for ci, ch in enumerate(chunks):
                        net = nets[ci]
                        for m in range(2):
                            ms = slice(m * P, (m + 1) * P)
                            nc.tensor.matmul(net[m][:], b1_sb[i][0][:, ms], hrs2[ci][0][:], start=False, stop=False)
                            nc.tensor.matmul(net[m][:], b1_sb[i][1][:, ms], hrs2[ci][1][:], start=False, stop=last)
                for ci, ch in enumerate(chunks):
                    net = nets[ci]
                    frs = []
                    for m in range(2):
                        fr = spool.tile([P, NF], F16, tag=f"fr{ci}_{m}", name=f"fr{ch}_{m}")
                        relu_op(cfg.get('fr', 'dd')[ci] == 'd', fr, net[m], rb_sb[:, 10 + m : 11 + m])
                        frs.append(fr)
                    op_ps = hpool.tile([1, NF], F32, tag="hps0", name=f"ops{ch}", bufs=2)
                    nc.tensor.matmul(op_ps[:], ow_sb[:, 0:1], frs[0][:], start=True, stop=False)
                    nc.tensor.matmul(op_ps[:], ow_sb[:, 1:2], frs[1][:], start=False, stop=True)
                    if cfg.get('outadd', 'v') == 'a':
                        nc.scalar.activation(
                            stage[:, ci * NF : (ci + 1) * NF], op_ps[:],
                            AF.Identity, bias=outb_sb[:1, :1], scale=1.0,
                        )
                    else:
                        nc.vector.tensor_scalar_add(
                            stage[:, ci * NF : (ci + 1) * NF], op_ps[:], outb_sb[:1, :1]
                        )
                nc.sync.dma_start(
                    out_dev[:, 2 * pc * NF : 2 * (pc + 1) * NF], stage[:]
                )

    nc.compile()
    return nc


def _build_table(grid_c):
    """grid_c: [C, 64, 64, 64] f32 (channels, z, y, x) -> [V8, ROW] fp16."""
    g = np.ascontiguousarray(np.transpose(grid_c, (1, 2, 3, 0))).astype(np.float16)
    gp = np.pad(g, ((0, 1), (0, 1), (0, 1), (0, 0)), mode="edge")  # [65,65,65,C]
    parts = []
    for sz in (0, 1):
        for sy in (0, 1):
            for sx in (0, 1):
                v = gp[sz : sz + 64, sy : sy + 64, sx : sx + 64]
                v = v.reshape(32, 2, 32, 2, 32, 2, C)
                v = np.ascontiguousarray(np.transpose(v, (0, 2, 4, 1, 3, 5, 6)))
                parts.append(v.reshape(VB, ROW))
    return np.concatenate(parts, axis=0)


def kernel(p, c_grid, fc_p_w, fc_p_b, fc_c_w, fc_c_b, b0_w, b0_b, b1_w, b1_b,
           out_w, out_b):
    p = np.asarray(p, np.float32)
    c_grid = np.asarray(c_grid, np.float32)
    fc_p_w = np.asarray(fc_p_w, np.float32)
    fc_p_b = np.asarray(fc_p_b, np.float32)
    fc_c_w = np.asarray(fc_c_w, np.float32)
    fc_c_b = np.asarray(fc_c_b, np.float32)
    b0_w = np.asarray(b0_w, np.float32)
    b0_b = np.asarray(b0_b, np.float32)
    b1_w = np.asarray(b1_w, np.float32)
    b1_b = np.asarray(b1_b, np.float32)
    out_w = np.asarray(out_w, np.float32)
    out_b = np.asarray(out_b, np.float32)

    if "nc" not in _CACHE:
        _CACHE["nc"] = _build_nc()
    nc = _CACHE["nc"]

    tables = [_build_table(c_grid[b]) for b in range(B)]

    # ---- weight prep (shared across cores) ----
    f16 = lambda a: np.ascontiguousarray(a).astype(np.float16)
    fcp = np.zeros((4, H), np.float32)
    fcp[:3] = fc_p_w.T
    fcp[3] = fc_p_b + fc_c_b[0]
    fcp = f16(fcp)
    wc = f16(np.transpose(fc_c_w, (0, 2, 1)))                       # [5,128,256]
    b0wt = f16(np.transpose(b0_w, (0, 2, 1)).reshape(NB, 2, P, H))  # K-tiles
    b1wt = f16(np.transpose(b1_w, (0, 2, 1)).reshape(NB, 2, P, H))
    oww = f16(out_w.reshape(H).reshape(2, P).T)                     # [128, 2]
    # cumulative missing-bias for relu views
    rbs = np.zeros((6, H), np.float32)
    acc = np.zeros(H, np.float32)
    for i in range(NB):
        if i > 0:
            acc = acc + fc_c_b[i]
        rbs[i] = acc
        acc = acc + b1_b[i]
    rbs[5] = acc
    rb_host = np.ascontiguousarray(
        rbs.reshape(6, 2, P).transpose(2, 0, 1).reshape(P, 12)
    ).astype(np.float32)
    b0b_host = np.ascontiguousarray(
        b0_b.reshape(NB, 2, P).transpose(2, 0, 1).reshape(P, 10)
    ).astype(np.float32)
    outb_host = np.asarray(out_b, np.float32).reshape(1, 1)

    in_maps = []
    for core in range(NCORES):
        b = core // CPB
        s = core % CPB
        sl = np.ascontiguousarray(p[b, s * NPTS : (s + 1) * NPTS])  # [NPTS, 3]
        v = sl.reshape(P, NCH, TPC, 3).transpose(3, 1, 2, 0)        # [3, 32, 4, 128]
        ptp = np.concatenate(
            [v.reshape(3, NPTS), np.ones((1, NPTS), np.float32)], axis=0
        ).astype(np.float16)
        in_maps.append(
            dict(table=tables[b], pts=sl, ptpad=np.ascontiguousarray(ptp),
                 fcp=fcp, wc=wc, b0w=b0wt, b1w=b1wt, oww=oww,
                 rb=rb_host, b0b=b0b_host, outb=outb_host)
        )

    res = run_bass_kernel_spmd(nc, in_maps, core_ids=list(range(NCORES)))

    out = np.empty((B, N, 1), np.float32)
    for core in range(NCORES):
        b = core // CPB
        s = core % CPB
        arr = res.results[core]["out_dev"][0]                       # [NPTS]
        a = arr.reshape(NCH, TPC, P).transpose(2, 0, 1).reshape(NPTS)
        out[b, s * NPTS : (s + 1) * NPTS, 0] = a
    return out


# revision 25
# speedup vs baseline: 1.4278x; 1.0289x over previous
"""Trainium2 Bass kernel for nn_LocalDecoder (ConvONet LocalDecoder: trilinear
grid sample + 5-block ResNet MLP decoder).

Strategy (8 NeuronCores):
  - Data-parallel over points: cores 0-3 take batch 0, cores 4-7 take batch 1,
    16384 points per core.
  - The feature grid is repacked on the host into an 8-shift 2x2x2-block table
    [8*32^3, 8*128] fp16: row (s, bz, by, bx) holds the 2x2x2 voxel block at
    alignment-shift s = (sz, sy, sx).  Every query point's 8 trilinear corners
    are then exactly ONE 2KB row -> one indirect-DMA descriptor per point.
  - Device computes voxel indices + trilinear weights on VectorE, gathers
    point-blocks via gpsimd indirect DMA (128 points/call), interpolates with
    fused scalar_tensor_tensor ops, transposes [pts,ch]->[ch,pts] on TensorE,
    and runs the MLP in fp16 with the residual stream resident in PSUM
    (fc_c / b1 matmuls accumulate in place; biases folded into ACT relu views).
"""

import numpy as np

import concourse.bass as bass
import concourse.bacc as bacc
import concourse.mybir as mybir
import concourse.tile as tile
from concourse.bass_utils import run_bass_kernel_spmd
from concourse.masks import make_identity

# ---- problem constants (hardcoded per contract) ----
B, N, R = 2, 65536, 64
C = 128            # grid feature channels
H = 256            # MLP hidden
NB = 5             # resnet blocks
PADDING = 0.1

NCORES = 8
CPB = NCORES // B          # cores per batch = 4
NPTS = N // CPB            # points per core = 16384
P = 128                    # partitions
T = NPTS // P              # 128 point-tiles of 128 per core
TPC = 4                    # tiles per chunk (chunk = 512 points)
NCH = T // TPC             # 32 chunks
NF = TPC * P               # chunk free dim = 512
VB = 32 * 32 * 32          # blocks per shift copy
V8 = 8 * VB                # table rows
ROW = 8 * C                # fp16 elems per table row (2KB)

SCALE = float(np.float32(63.0) / np.float32(1.0 + PADDING + 1e-3))
OFF = 31.5

F16 = mybir.dt.float16
F32 = mybir.dt.float32
I32 = mybir.dt.int32
ALU = mybir.AluOpType
AF = mybir.ActivationFunctionType

_CACHE = {}

DEFAULT_CFG = dict(rin="aa", hr="dddd", fr="ad", outadd="a", trcopy="a", gsplit=4,
                   cbufs=3, gbufs=3, sbufs=3)


def _build_nc(cfg=None):
    cfg = cfg if cfg is not None else DEFAULT_CFG
    nc = bacc.Bacc("TRN2", target_bir_lowering=False, debug=False)

    table = nc.dram_tensor("table", [V8, ROW], F16, kind="ExternalInput")
    pts = nc.dram_tensor("pts", [NPTS, 3], F32, kind="ExternalInput")
    ptpad = nc.dram_tensor("ptpad", [4, NPTS], F16, kind="ExternalInput")
    fcp = nc.dram_tensor("fcp", [4, H], F16, kind="ExternalInput")
    wc = nc.dram_tensor("wc", [NB, C, H], F16, kind="ExternalInput")
    b0w = nc.dram_tensor("b0w", [NB, 2, P, H], F16, kind="ExternalInput")
    b1w = nc.dram_tensor("b1w", [NB, 2, P, H], F16, kind="ExternalInput")
    oww = nc.dram_tensor("oww", [P, 2], F16, kind="ExternalInput")
    rb = nc.dram_tensor("rb", [P, 12], F32, kind="ExternalInput")
    b0b = nc.dram_tensor("b0b", [P, 10], F32, kind="ExternalInput")
    outb = nc.dram_tensor("outb", [1, 1], F32, kind="ExternalInput")
    out_dev = nc.dram_tensor("out_dev", [1, NPTS], F32, kind="ExternalOutput")

    with tile.TileContext(nc) as tc:
        with (
            tc.tile_pool(name="const", bufs=1) as kpool,
            tc.tile_pool(name="gather", bufs=cfg.get("gbufs", 2)) as gpool,
            tc.tile_pool(name="feat", bufs=cfg.get("fbufs", 3)) as fpool,
            tc.tile_pool(name="cs", bufs=cfg.get("cbufs", 2)) as cpool,
            tc.tile_pool(name="act", bufs=cfg.get("sbufs", 2)) as spool,
            tc.tile_pool(name="pp", bufs=2) as ppool,
            tc.tile_pool(name="stage", bufs=2) as stpool,
            tc.tile_pool(name="net_ps", bufs=1, space="PSUM") as npool,
            tc.tile_pool(name="h_ps", bufs=1, space="PSUM") as hpool,
            tc.tile_pool(name="tr_ps", bufs=1, space="PSUM") as trpool,
            tc.tile_pool(name="o_ps", bufs=1, space="PSUM") as opool,
        ):
            # ---------- point slab first: it gates idx -> gathers ----------
            p_slab = kpool.tile([P, T * 3], F32, tag="pslab")
            nc.sync.dma_start(
                p_slab[:], pts[:].rearrange("(p t) c -> p (t c)", p=P)
            )
            # ---------- load constants (ACT HWDGE queue; don't block SP) ----
            fcp_sb = kpool.tile([4, H], F16, tag="fcp")
            nc.scalar.dma_start(fcp_sb[:], fcp[:])
            wc_sb = []
            b0_sb = []
            b1_sb = []
            for i in range(NB):
                w = kpool.tile([P, H], F16, tag=f"wc{i}")
                nc.scalar.dma_start(w[:], wc[i, :, :])
                wc_sb.append(w)
                r0 = []
                r1 = []
                for kk in range(2):
                    a = kpool.tile([P, H], F16, tag=f"b0_{i}_{kk}")
                    nc.scalar.dma_start(a[:], b0w[i, kk, :, :])
                    r0.append(a)
                    b = kpool.tile([P, H], F16, tag=f"b1_{i}_{kk}")
                    nc.scalar.dma_start(b[:], b1w[i, kk, :, :])
                    r1.append(b)
                b0_sb.append(r0)
                b1_sb.append(r1)
            ow_sb = kpool.tile([P, 2], F16, tag="oww")
            nc.scalar.dma_start(ow_sb[:], oww[:])
            rb_sb = kpool.tile([P, 12], F32, tag="rb")
            nc.scalar.dma_start(rb_sb[:], rb[:])
            b0b_sb = kpool.tile([P, 10], F32, tag="b0b")
            nc.scalar.dma_start(b0b_sb[:], b0b[:])
            outb_sb = kpool.tile([1, 1], F32, tag="outb")
            nc.scalar.dma_start(outb_sb[:], outb[:])
            ident = kpool.tile([P, P], F16, tag="ident")
            make_identity(nc, ident[:])

            # ---------- index / weight precompute (VectorE) ----------
            # p_slab[p, t*3+c] = pts[p*T + t, c].  Emitted in two column
            # ranges (first pair's tiles, then the rest) so the pipeline can
            # start gathering immediately.
            ix = kpool.tile([P, T * 3], F32, tag="ix")
            x0i = kpool.tile([P, T * 3], I32, tag="x0i")
            x0f = kpool.tile([P, T * 3], F32, tag="x0f")
            fixm = kpool.tile([P, T * 3], F32, tag="fixm")
            w_all = kpool.tile([P, T * 3], F32, tag="wall")
            u_all = kpool.tile([P, T * 3], F32, tag="uall")
            dimt = {}
            for d in range(3):
                for nm in ("half", "hi", "hf", "m2", "sd"):
                    dt_ = I32 if nm == "hi" else F32
                    dimt[(nm, d)] = kpool.tile([P, T], dt_, tag=f"{nm}{d}", name=f"{nm}{d}")
            t1 = kpool.tile([P, T], F32, tag="t1")
            t3 = kpool.tile([P, T], F32, tag="t3")
            rr = kpool.tile([P, T], F32, tag="rr")
            idx_sb = kpool.tile([P, T], I32, tag="idx")
            w8 = [kpool.tile([P, T], F32, tag=f"w8_{k}", name=f"w8_{k}") for k in range(8)]

            def emit_idx(lo, hi):
                s3 = slice(lo * 3, hi * 3)
                s1 = slice(lo, hi)
                nc.vector.tensor_scalar(ix[:, s3], p_slab[:, s3], SCALE, OFF, op0=ALU.mult, op1=ALU.add)
                nc.vector.tensor_scalar_max(ix[:, s3], ix[:, s3], 0.0)
                nc.vector.tensor_scalar_min(ix[:, s3], ix[:, s3], 63.0)
                nc.vector.tensor_copy(x0i[:, s3], ix[:, s3])
                nc.vector.tensor_copy(x0f[:, s3], x0i[:, s3])
                nc.vector.tensor_tensor(out=fixm[:, s3], in0=x0f[:, s3], in1=ix[:, s3], op=ALU.is_gt)
                nc.vector.tensor_tensor(out=x0f[:, s3], in0=x0f[:, s3], in1=fixm[:, s3], op=ALU.subtract)
                nc.vector.tensor_scalar_min(x0f[:, s3], x0f[:, s3], 62.0)
                nc.vector.tensor_tensor(out=w_all[:, s3], in0=ix[:, s3], in1=x0f[:, s3], op=ALU.subtract)
                nc.vector.tensor_scalar(u_all[:, s3], w_all[:, s3], -1.0, 1.0, op0=ALU.mult, op1=ALU.add)

                x0v = x0f[:].rearrange("p (t c) -> p t c", c=3)
                bds = []
                sds = []
                for d in range(3):
                    xv = x0v[:, s1, d]
                    half, hi_, hf, m2, sd = (dimt[(nm, d)] for nm in ("half", "hi", "hf", "m2", "sd"))
                    nc.vector.tensor_scalar_mul(half[:, s1], xv, 0.5)
                    nc.vector.tensor_copy(hi_[:, s1], half[:, s1])
                    nc.vector.tensor_copy(hf[:, s1], hi_[:, s1])
                    nc.vector.tensor_tensor(out=m2[:, s1], in0=hf[:, s1], in1=half[:, s1], op=ALU.is_gt)
                    nc.vector.tensor_tensor(out=hf[:, s1], in0=hf[:, s1], in1=m2[:, s1], op=ALU.subtract)
                    nc.vector.scalar_tensor_tensor(
                        out=sd[:, s1], in0=hf[:, s1], scalar=-2.0, in1=xv, op0=ALU.mult, op1=ALU.add
                    )
                    bds.append(hf)
                    sds.append(sd)
                bx, by, bz = bds
                sx, sy, sz = sds
                nc.vector.scalar_tensor_tensor(out=t1[:, s1], in0=sz[:, s1], scalar=2.0, in1=sy[:, s1], op0=ALU.mult, op1=ALU.add)
                nc.vector.scalar_tensor_tensor(out=t1[:, s1], in0=t1[:, s1], scalar=2.0, in1=sx[:, s1], op0=ALU.mult, op1=ALU.add)
                nc.vector.scalar_tensor_tensor(out=t3[:, s1], in0=bz[:, s1], scalar=32.0, in1=by[:, s1], op0=ALU.mult, op1=ALU.add)
                nc.vector.scalar_tensor_tensor(out=t3[:, s1], in0=t3[:, s1], scalar=32.0, in1=bx[:, s1], op0=ALU.mult, op1=ALU.add)
                nc.vector.scalar_tensor_tensor(out=rr[:, s1], in0=t1[:, s1], scalar=float(VB), in1=t3[:, s1], op0=ALU.mult, op1=ALU.add)
                nc.vector.tensor_copy(idx_sb[:, s1], rr[:, s1])

                # 8 corner-weight products W8[k][p, t], k = dz*4 + dy*2 + dx
                wv = w_all[:].rearrange("p (t c) -> p t c", c=3)
                uv = u_all[:].rearrange("p (t c) -> p t c", c=3)
                for k in range(8):
                    dz, dy, dx = (k >> 2) & 1, (k >> 1) & 1, k & 1
                    zf = (wv if dz else uv)[:, s1, 2]
                    yf = (wv if dy else uv)[:, s1, 1]
                    xf = (wv if dx else uv)[:, s1, 0]
                    wk = w8[k]
                    nc.vector.tensor_tensor(out=wk[:, s1], in0=zf, in1=yf, op=ALU.mult)
                    nc.vector.tensor_tensor(out=wk[:, s1], in0=wk[:, s1], in1=xf, op=ALU.mult)

            stages = cfg.get('stages', (2 * TPC, 4 * TPC, 8 * TPC, T))
            lo = 0
            for hi_t in stages:
                emit_idx(lo, hi_t)
                lo = hi_t

            # ---------- main loop: chunk PAIRS, MLPs interleaved ----------
            # Two independent per-chunk dependency chains fill each other's
            # engine stalls; relu engine alternates by chunk parity so the
            # two chains mostly use disjoint engines (ACT vs DVE).
            def relu_op(on_dve, dst, src, bias_ap, split=False):
                if split:
                    hf = NF // 2
                    nc.scalar.activation(
                        dst[:, :hf], src[:, :hf], AF.Relu, bias=bias_ap, scale=1.0
                    )
                    nc.vector.tensor_scalar(
                        dst[:, hf:], src[:, hf:], bias_ap, 0.0, op0=ALU.add, op1=ALU.max
                    )
                elif not on_dve:
                    nc.scalar.activation(dst[:], src[:], AF.Relu, bias=bias_ap, scale=1.0)
                else:
                    nc.vector.tensor_scalar(
                        dst[:], src[:], bias_ap, 0.0, op0=ALU.add, op1=ALU.max
                    )

            for pc in range(NCH // 2):
                chunks = (2 * pc, 2 * pc + 1)
                ptp_pair = ppool.tile([4, 2 * NF], F16, tag="ptpp", name=f"ptpp{pc}")
                nc.sync.dma_start(
                    ptp_pair[:], ptpad[:, 2 * pc * NF : 2 * (pc + 1) * NF]
                )
                stage = stpool.tile([1, 2 * NF], F32, tag="stage", name=f"stage{pc}")
                csbs = []
                tr_ps = trpool.tile([P, 2 * TPC, P], F16, tag="trps", name=f"trps{pc}")
                gts = {}
                for ci, ch in enumerate(chunks):
                    for tl in range(TPC):
                        t = TPC * ch + tl
                        g = gpool.tile([P, ROW], F16, tag=f"g{ci}_{tl}", name=f"g{ch}_{tl}")
                        nc.gpsimd.indirect_dma_start(
                            out=g[:],
                            out_offset=None,
                            in_=table[:],
                            in_offset=bass.IndirectOffsetOnAxis(
                                ap=idx_sb[:, t : t + 1], axis=0
                            ),
                        )
                        gts[(ci, tl)] = g
                gsplit = cfg.get("gsplit", 2)
                for ci, ch in enumerate(chunks):
                    for tl in range(TPC):
                        t = TPC * ch + tl
                        g = gts[(ci, tl)]
                        eng = nc.vector if tl < gsplit else nc.gpsimd
                        facc = fpool.tile([P, P], F16, tag=f"fa{ci}_{tl}", name=f"fa{ch}_{tl}")
                        eng.tensor_scalar_mul(
                            facc[:], g[:, 0:C], w8[0][:, t : t + 1]
                        )
                        for k in range(1, 8):
                            eng.scalar_tensor_tensor(
                                out=facc[:],
                                in0=g[:, k * C : (k + 1) * C],
                                scalar=w8[k][:, t : t + 1],
                                in1=facc[:],
                                op0=ALU.mult,
                                op1=ALU.add,
                            )
                        nc.tensor.transpose(tr_ps[:, ci * TPC + tl, :], facc[:], ident[:])
                    c_sb = cpool.tile([P, NF], F16, tag=f"csb{ci}", name=f"csb{ch}")
                    if cfg.get('trcopy', 'v') == 'a':
                        nc.scalar.copy(c_sb[:], tr_ps[:, ci * TPC : (ci + 1) * TPC, :])
                    else:
                        nc.vector.tensor_copy(c_sb[:], tr_ps[:, ci * TPC : (ci + 1) * TPC, :])
                    csbs.append(c_sb)

                # ----- interleaved MLPs: residual streams live in PSUM -----
                nets = []
                for ci, ch in enumerate(chunks):
                    net = [
                        npool.tile([P, NF], F32, tag=f"net{ci}_{m}", name=f"net{ch}_{m}")
                        for m in range(2)
                    ]
                    for m in range(2):
                        ms = slice(m * P, (m + 1) * P)
                        nc.tensor.matmul(
                            net[m][:], fcp_sb[:, ms],
                            ptp_pair[:, ci * NF : (ci + 1) * NF],
                            start=True, stop=False,
                        )
                    nets.append(net)
                for i in range(NB):
                    rins2 = []
                    for ci, ch in enumerate(chunks):
                        net = nets[ci]
                        for m in range(2):
                            ms = slice(m * P, (m + 1) * P)
                            nc.tensor.matmul(
                                net[m][:], wc_sb[i][:, ms], csbs[ci][:], start=False, stop=False
                            )
                        rins = []
                        for m in range(2):
                            r = spool.tile([P, NF], F16, tag=f"rin{ci}_{m}", name=f"rin{ch}_{m}")
                            relu_op(cfg.get('rin', 'aa')[ci] == 'd', r, net[m], rb_sb[:, 2 * i + m : 2 * i + m + 1], split=cfg.get('split_rin', False))
                            rins.append(r)
                        rins2.append(rins)
                    hrs2 = []
                    for ci, ch in enumerate(chunks):
                        hrs = []
                        for m in range(2):
                            ms = slice(m * P, (m + 1) * P)
                            hp = hpool.tile([P, NF], F32, tag=f"hps{m}", name=f"hps{ch}_{m}", bufs=(2 if m == 0 else 1))
                            nc.tensor.matmul(hp[:], b0_sb[i][0][:, ms], rins2[ci][0][:], start=True, stop=False)
                            nc.tensor.matmul(hp[:], b0_sb[i][1][:, ms], rins2[ci][1][:], start=False, stop=True)
                            hr = spool.tile([P, NF], F16, tag=f"hr{ci}_{m}", name=f"hr{ch}_{m}")
                            relu_op((cfg['hrb'][i] == 'd') if 'hrb' in cfg else cfg.get('hr', 'adad')[2 * ci + m] == 'd', hr, hp, b0b_sb[:, 2 * i + m : 2 * i + m + 1], split=cfg.get('split_hr', False))
                            hrs.append(hr)
                        hrs2.append(hrs)
                    last = i == NB - 1
                    for ci, ch in enumerate(chunks):
                        net = nets[ci]
                        for m in range(2):
                            ms = slice(m * P, (m + 1) * P)
                            nc.tensor.matmul(net[m][:], b1_sb[i][0][:, ms], hrs2[ci][0][:], start=False, stop=False)
                            nc.tensor.matmul(net[m][:], b1_sb[i][1][:, ms], hrs2[ci][1][:], start=False, stop=last)
                for ci, ch in enumerate(chunks):
                    net = nets[ci]
                    frs = []
                    for m in range(2):
                        fr = spool.tile([P, NF], F16, tag=f"fr{ci}_{m}", name=f"fr{ch}_{m}")
                        relu_op(cfg.get('fr', 'dd')[ci] == 'd', fr, net[m], rb_sb[:, 10 + m : 11 + m])
                        frs.append(fr)
                    op_ps = hpool.tile([1, NF], F32, tag="hps0", name=f"ops{ch}", bufs=2)
                    nc.tensor.matmul(op_ps[:], ow_sb[:, 0:1], frs[0][:], start=True, stop=False)
                    nc.tensor.matmul(op_ps[:], ow_sb[:, 1:2], frs[1][:], start=False, stop=True)
                    if cfg.get('outadd', 'v') == 'a':
                        nc.scalar.activation(
                            stage[:, ci * NF : (ci + 1) * NF], op_ps[:],
                            AF.Identity, bias=outb_sb[:1, :1], scale=1.0,
                        )
                    else:
                        nc.vector.tensor_scalar_add(
                            stage[:, ci * NF : (ci + 1) * NF], op_ps[:], outb_sb[:1, :1]
                        )
                nc.sync.dma_start(
                    out_dev[:, 2 * pc * NF : 2 * (pc + 1) * NF], stage[:]
                )

    nc.compile()
    return nc


def _build_table(grid_c):
    """grid_c: [C, 64, 64, 64] f32 (channels, z, y, x) -> [V8, ROW] fp16."""
    g = np.ascontiguousarray(np.transpose(grid_c, (1, 2, 3, 0))).astype(np.float16)
    gp = np.pad(g, ((0, 1), (0, 1), (0, 1), (0, 0)), mode="edge")  # [65,65,65,C]
    parts = []
    for sz in (0, 1):
        for sy in (0, 1):
            for sx in (0, 1):
                v = gp[sz : sz + 64, sy : sy + 64, sx : sx + 64]
                v = v.reshape(32, 2, 32, 2, 32, 2, C)
                v = np.ascontiguousarray(np.transpose(v, (0, 2, 4, 1, 3, 5, 6)))
                parts.append(v.reshape(VB, ROW))
    return np.concatenate(parts, axis=0)


def kernel(p, c_grid, fc_p_w, fc_p_b, fc_c_w, fc_c_b, b0_w, b0_b, b1_w, b1_b,
           out_w, out_b):
    p = np.asarray(p, np.float32)
    c_grid = np.asarray(c_grid, np.float32)
    fc_p_w = np.asarray(fc_p_w, np.float32)
    fc_p_b = np.asarray(fc_p_b, np.float32)
    fc_c_w = np.asarray(fc_c_w, np.float32)
    fc_c_b = np.asarray(fc_c_b, np.float32)
    b0_w = np.asarray(b0_w, np.float32)
    b0_b = np.asarray(b0_b, np.float32)
    b1_w = np.asarray(b1_w, np.float32)
    b1_b = np.asarray(b1_b, np.float32)
    out_w = np.asarray(out_w, np.float32)
    out_b = np.asarray(out_b, np.float32)

    if "nc" not in _CACHE:
        _CACHE["nc"] = _build_nc()
    nc = _CACHE["nc"]

    tables = [_build_table(c_grid[b]) for b in range(B)]

    # ---- weight prep (shared across cores) ----
    f16 = lambda a: np.ascontiguousarray(a).astype(np.float16)
    fcp = np.zeros((4, H), np.float32)
    fcp[:3] = fc_p_w.T
    fcp[3] = fc_p_b + fc_c_b[0]
    fcp = f16(fcp)
    wc = f16(np.transpose(fc_c_w, (0, 2, 1)))                       # [5,128,256]
    b0wt = f16(np.transpose(b0_w, (0, 2, 1)).reshape(NB, 2, P, H))  # K-tiles
    b1wt = f16(np.transpose(b1_w, (0, 2, 1)).reshape(NB, 2, P, H))
    oww = f16(out_w.reshape(H).reshape(2, P).T)                     # [128, 2]
    # cumulative missing-bias for relu views
    rbs = np.zeros((6, H), np.float32)
    acc = np.zeros(H, np.float32)
    for i in range(NB):
        if i > 0:
            acc = acc + fc_c_b[i]
        rbs[i] = acc
        acc = acc + b1_b[i]
    rbs[5] = acc
    rb_host = np.ascontiguousarray(
        rbs.reshape(6, 2, P).transpose(2, 0, 1).reshape(P, 12)
    ).astype(np.float32)
    b0b_host = np.ascontiguousarray(
        b0_b.reshape(NB, 2, P).transpose(2, 0, 1).reshape(P, 10)
    ).astype(np.float32)
    outb_host = np.asarray(out_b, np.float32).reshape(1, 1)

    in_maps = []
    for core in range(NCORES):
        b = core // CPB
        s = core % CPB
        sl = np.ascontiguousarray(p[b, s * NPTS : (s + 1) * NPTS])  # [NPTS, 3]
        v = sl.reshape(P, NCH, TPC, 3).transpose(3, 1, 2, 0)        # [3, 32, 4, 128]
        ptp = np.concatenate(
            [v.reshape(3, NPTS), np.ones((1, NPTS), np.float32)], axis=0
        ).astype(np.float16)
        in_maps.append(
            dict(table=tables[b], pts=sl, ptpad=np.ascontiguousarray(ptp),
                 fcp=fcp, wc=wc, b0w=b0wt, b1w=b1wt, oww=oww,
                 rb=rb_host, b0b=b0b_host, outb=outb_host)
        )

    res = run_bass_kernel_spmd(nc, in_maps, core_ids=list(range(NCORES)))

    out = np.empty((B, N, 1), np.float32)
    for core in range(NCORES):
        b = core // CPB
        s = core % CPB
        arr = res.results[core]["out_dev"][0]                       # [NPTS]
        a = arr.reshape(NCH, TPC, P).transpose(2, 0, 1).reshape(NPTS)
        out[b, s * NPTS : (s + 1) * NPTS, 0] = a
    return out
